# revision 6
# baseline (speedup 1.0000x reference)
"""Trainium2 Bass kernel for nn_CLVModel (GNN message passing + LSTM cell + MLP head).

Sharding: destination-sharded across 8 NeuronCores, no collectives.
  - customers split into 8 shards of 25088 rows (196 chunks x 128)
  - products split into 8 shards of 5120 rows (40 chunks x 128)
  - rev edges  (product -> customer) assigned to the core owning dst customer
  - buys edges (customer -> product) assigned to the core owning dst product
Each core: for each dst chunk (128 nodes), gathers h_src rows via indirect DMA
(128 rows per instruction), builds a one-hot dst-selection matrix on DVE/ACT,
and accumulates segment sums on the PE into PSUM (S^T @ V).  Mean-scaling by
reciprocal in-degree is fused into the PSUM->SBUF flush.  Phase B computes the
four LSTM gate pre-activations as node-major matmuls (K = features|agg|h),
with the country embedding folded into the K dimension as a one-hot against a
device-computed (country_emb @ W_emb) projection, applies the LSTM cell
update, and the 2-layer prediction MLP for customers.
"""

import numpy as np

# ---------------------------------------------------------------- constants
NC, NP, E = 200_000, 40_000, 1_600_000
OUT = 64
N_COUNTRY, D_COUNTRY = 38, 4
N_DESC, D_DESC = 4000, 16
GATES = ("i", "f", "g", "o")
NCORES = 8

NCS = 25088          # customers per core (196 chunks of 128)
NPS = 5120           # products per core (40 chunks of 128)
NCC = NCS // 128     # 196
NPC = NPS // 128     # 40
NC_PAD = NCS * NCORES   # 200704
NP_PAD = NPS * NCORES   # 40960

F32 = None  # set after mybir import
_CACHE = {}
_LAST_IN_MAPS = None


# ---------------------------------------------------------------- wait split
def _split_excess_waits(nc, max_waits=1):
    """This walrus build rejects >1 sem-wait per instruction; move excess
    waits onto standalone EventSemaphore instructions just before."""
    import concourse.mybir as mybir

    ctr = [0]
    for f in nc.m.functions:
        for bb in f.blocks:
            insts = list(bb.instructions)
            out = []
            changed = False
            for inst in insts:
                si = inst.sync_info
                if si is not None:
                    waits = list(si.on_wait)
                    if len(waits) > max_waits:
                        changed = True
                        extra, keep = waits[:-max_waits], waits[-max_waits:]
                        for i in range(0, len(extra), max_waits):
                            chunk = extra[i:i + max_waits]
                            ctr[0] += 1
                            n = mybir.InstEventSemaphore(
                                name=f"I-waitsplit-{ctr[0]}", ins=[], outs=[])
                            n.engine = inst.engine
                            n.sync_info = mybir.SyncInfo(
                                on_wait=chunk, on_update=[])
                            out.append(n)
                        inst.sync_info = mybir.SyncInfo(
                            on_wait=keep, on_update=list(si.on_update))
                out.append(inst)
            if changed:
                bb.instructions = out


def _patch_bass():
    import concourse.bass as bass
    if getattr(bass.Bass, "_clv_patched", False):
        return
    orig = bass.Bass.to_json_bytes

    def patched(self, *a, **k):
        _split_excess_waits(self)
        return orig(self, *a, **k)

    bass.Bass.to_json_bytes = patched
    bass.Bass._clv_patched = True


# ---------------------------------------------------------------- runner
class _Runner:
    """Compile once via PJRT (axon) and execute with 8-way shard_map."""

    def __init__(self, nc, n_cores):
        import jax
        import concourse.mybir as mybir
        from concourse.bass2jax import (
            _bass_exec_p, install_neuronx_cc_hook, partition_id_tensor)
        from jax.sharding import Mesh, PartitionSpec
        from jax.experimental.shard_map import shard_map

        install_neuronx_cc_hook()
        self.jax = jax
        partition_name = (nc.partition_id_tensor.name
                          if nc.partition_id_tensor else None)
        self.in_names, self.out_names = [], []
        self.out_avals, self.zero_outs = [], []
        for alloc in nc.m.functions[0].allocations:
            if not isinstance(alloc, mybir.MemoryLocationSet):
                continue
            name = alloc.memorylocations[0].name
            if alloc.kind == "ExternalInput":
                if name != partition_name:
                    self.in_names.append(name)
            elif alloc.kind == "ExternalOutput":
                self.out_names.append(name)
                shape = tuple(alloc.tensor_shape)
                dtype = mybir.dt.np(alloc.dtype)
                self.out_avals.append(jax.core.ShapedArray(shape, dtype))
                self.zero_outs.append(np.zeros(shape, dtype))
        n_params = len(self.in_names)
        n_outs = len(self.out_avals)
        all_in = self.in_names + self.out_names + (
            [partition_name] if partition_name else [])
        self.n_cores = n_cores
        out_avals = tuple(self.out_avals)

        def _body(*args):
            operands = list(args)
            if partition_name is not None:
                operands.append(partition_id_tensor())
            outs = _bass_exec_p.bind(
                *operands, out_avals=out_avals, in_names=tuple(all_in),
                out_names=tuple(self.out_names),
                lowering_input_output_aliases=(),
                sim_require_finite=False, sim_require_nnan=False, nc=nc)
            return tuple(outs)

        devices = jax.devices()[:n_cores]
        mesh = Mesh(np.asarray(devices), ("core",))
        in_specs = (PartitionSpec("core"),) * (n_params + n_outs)
        out_specs = (PartitionSpec("core"),) * n_outs
        self.sharded = jax.jit(
            shard_map(_body, mesh=mesh, in_specs=in_specs,
                      out_specs=out_specs, check_rep=False),
            keep_unused=True)

    def run(self, in_maps):
        jax = self.jax
        per_core = [[np.asarray(m[nm]) for nm in self.in_names]
                    for m in in_maps]
        concat_in = [
            np.concatenate([per_core[c][i] for c in range(self.n_cores)],
                           axis=0)
            for i in range(len(self.in_names))]
        concat_zeros = [
            np.zeros((self.n_cores * z.shape[0], *z.shape[1:]), z.dtype)
            for z in self.zero_outs]
        out = self.sharded(*concat_in, *concat_zeros)
        jax.block_until_ready(out)
        return [
            {nm: np.asarray(out[i]).reshape(
                self.n_cores, *self.out_avals[i].shape)[c]
             for i, nm in enumerate(self.out_names)}
            for c in range(self.n_cores)
        ]


# ---------------------------------------------------------------- host prep
def _prep_edges(src, dst, shard_rows, n_chunks):
    """Bucket edges by dst shard/chunk, pad each chunk's edge list to a
    multiple of 128 slots with a tile count shared across all 8 cores.

    Returns (per_core [(src_pm [128,NT], dstl_pm [128,NT])], nt [n_chunks],
             rcnts per core [128, n_chunks])."""
    core = dst // shard_rows
    dstl = dst - core * shard_rows
    ch = dstl // 128
    dloc = (dstl % 128).astype(np.float32)
    cnts = np.zeros((NCORES, n_chunks), np.int64)
    per_core_e = []
    for k in range(NCORES):
        m = core == k
        per_core_e.append((src[m].astype(np.int32), ch[m], dloc[m], dstl[m]))
        cnts[k] = np.bincount(ch[m], minlength=n_chunks)
    nt = np.maximum(1, -(-cnts.max(axis=0) // 128))
    starts = np.concatenate([[0], np.cumsum(nt)]).astype(np.int64)
    S = int(starts[-1]) * 128
    outs, rcnts = [], []
    for k in range(NCORES):
        s_k, ch_k, dl_k, dstl_k = per_core_e[k]
        order = np.argsort(ch_k, kind="stable")
        s_k, ch_k, dl_k = s_k[order], ch_k[order], dl_k[order]
        cstart = np.concatenate([[0], np.cumsum(cnts[k])]).astype(np.int64)
        pos = np.arange(len(ch_k), dtype=np.int64) - cstart[ch_k]
        slot = starts[ch_k] * 128 + pos
        src_slots = np.zeros(S, np.int32)
        dstl_slots = np.full(S, -1.0, np.float32)
        src_slots[slot] = s_k
        dstl_slots[slot] = dl_k
        outs.append((np.ascontiguousarray(src_slots.reshape(-1, 128).T),
                     np.ascontiguousarray(dstl_slots.reshape(-1, 128).T)))
        cnt_dst = np.bincount(dstl_k, minlength=shard_rows)
        rc = (1.0 / np.maximum(cnt_dst, 1)).astype(np.float32)
        rcnts.append(np.ascontiguousarray(rc.reshape(-1, 128).T))
    return outs, nt, starts, rcnts


def _stack_g(arrs):
    return np.concatenate([np.asarray(a, np.float32) for a in arrs], axis=1)


# ---------------------------------------------------------------- kernel IR
def _build_nc(nt_rev, st_rev, nt_buys, st_buys):
    import concourse.bass as bass
    import concourse.mybir as mybir
    import concourse.tile as tile
    from concourse.masks import make_identity

    _patch_bass()
    f32 = mybir.dt.float32
    i32 = mybir.dt.int32
    NT_REV = int(st_rev[-1])
    NT_BUYS = int(st_buys[-1])

    nc = bass.Bass()
    dp = nc.declare_dram_parameter

    # full tables for gathers (padded), per-core shards for phase B
    h_c_full = dp("h_c_full", [NC_PAD, OUT], f32, isOutput=False)
    h_p_full = dp("h_p_full", [NP_PAD, OUT], f32, isOutput=False)
    h_c_sh = dp("h_c_sh", [NCS, OUT], f32, isOutput=False)
    h_p_sh = dp("h_p_sh", [NPS, OUT], f32, isOutput=False)
    c_c_sh = dp("c_c_sh", [NCS, OUT], f32, isOutput=False)
    c_p_sh = dp("c_p_sh", [NPS, OUT], f32, isOutput=False)
    x_c_sh = dp("x_c_sh", [NCS, 15], f32, isOutput=False)
    x_p_sh = dp("x_p_sh", [NPS, 5], f32, isOutput=False)
    pid_d = dp("pid", [128, NPC], i32, isOutput=False)
    desc_emb_d = dp("desc_emb", [N_DESC, D_DESC], f32, isOutput=False)
    cemb_d = dp("country_emb", [N_COUNTRY, D_COUNTRY], f32, isOutput=False)

    rev_src_d = dp("rev_src", [128, NT_REV], i32, isOutput=False)
    rev_dstl_d = dp("rev_dstl", [128, NT_REV], f32, isOutput=False)
    rcnt_c_d = dp("rcnt_c", [128, NCC], f32, isOutput=False)
    buys_src_d = dp("buys_src", [128, NT_BUYS], i32, isOutput=False)
    buys_dstl_d = dp("buys_dstl", [128, NT_BUYS], f32, isOutput=False)
    rcnt_p_d = dp("rcnt_p", [128, NPC], f32, isOutput=False)

    wc_a_d = dp("wc_a", [15, 256], f32, isOutput=False)      # feat + b row
    wc_emb_d = dp("wc_emb", [4, 256], f32, isOutput=False)
    bl_c_d = dp("bl_c", [1, 256], f32, isOutput=False)
    wl_c_d = dp("wl_c", [64, 256], f32, isOutput=False)
    wr_c_d = dp("wr_c", [64, 256], f32, isOutput=False)
    wp_d = dp("wp", [22, 256], f32, isOutput=False)
    wl_p_d = dp("wl_p", [64, 256], f32, isOutput=False)
    wr_p_d = dp("wr_p", [64, 256], f32, isOutput=False)
    w1_d = dp("w1", [64, 32], f32, isOutput=False)
    b1_d = dp("b1", [32, 1], f32, isOutput=False)
    w2b_d = dp("w2b", [33, 1], f32, isOutput=False)

    pred_o = dp("pred_o", [NCS, 1], f32, isOutput=True)
    hc_o = dp("hc_o", [NCS, OUT], f32, isOutput=True)
    cc_o = dp("cc_o", [NCS, OUT], f32, isOutput=True)
    hp_o = dp("hp_o", [NPS, OUT], f32, isOutput=True)
    cp_o = dp("cp_o", [NPS, OUT], f32, isOutput=True)

    AF = mybir.ActivationFunctionType
    IOA = bass.IndirectOffsetOnAxis

    with tile.TileContext(nc) as tc:
        with (
            tc.tile_pool(name="const", bufs=1) as cpool,
            tc.tile_pool(name="arena", bufs=1) as apool,
            tc.tile_pool(name="vwork", bufs=32) as vpool,
            tc.tile_pool(name="soh", bufs=16) as spool,
            tc.tile_pool(name="bwork", bufs=3) as bpool,
            tc.tile_pool(name="edge_ps", bufs=3, space="PSUM") as eps,
            tc.tile_pool(name="pre_ps", bufs=2, space="PSUM") as pps,
            tc.tile_pool(name="tp_ps", bufs=3, space="PSUM") as tps,
        ):
            # ---------------- startup constants
            ident = cpool.tile([128, 128], f32)
            make_identity(nc, ident[:])
            iota128 = cpool.tile([128, 128], f32)
            nc.gpsimd.iota(iota128[:], pattern=[[1, 128]], base=0,
                           channel_multiplier=0,
                           allow_small_or_imprecise_dtypes=True)

            def load_const(dram, shape):
                t = cpool.tile(shape, f32, tag=dram.name)
                nc.sync.dma_start(t[:], dram[:])
                return t

            wl_c = load_const(wl_c_d, [64, 256])
            wr_c = load_const(wr_c_d, [64, 256])
            wp = load_const(wp_d, [22, 256])
            wl_p = load_const(wl_p_d, [64, 256])
            wr_p = load_const(wr_p_d, [64, 256])
            w1 = load_const(w1_d, [64, 32])
            b1 = load_const(b1_d, [32, 1])
            w2b = load_const(w2b_d, [33, 1])
            wcemb = load_const(wc_emb_d, [4, 256])
            cemb = load_const(cemb_d, [N_COUNTRY, D_COUNTRY])

            # customer W arena [54, 256]: 0:14 feat | 14 b | 15:53 proj | 53 bl
            wc_ar = cpool.tile([54, 256], f32)
            nc.sync.dma_start(wc_ar[0:15, :], wc_a_d[:])
            nc.sync.dma_start(wc_ar[53:54, :], bl_c_d[:])
            # proj = country_emb @ wc_emb  (on device)
            ps_e = tps.tile([D_COUNTRY, N_COUNTRY], f32, tag="tp")
            nc.tensor.transpose(ps_e[:], cemb[:],
                                ident[0:N_COUNTRY, 0:N_COUNTRY])
            ceT = bpool.tile([D_COUNTRY, N_COUNTRY], f32, tag="ceT")
            nc.any.tensor_copy(ceT[:], ps_e[:])
            ps_proj = pps.tile([N_COUNTRY, 256], f32, tag="pre")
            nc.tensor.matmul(ps_proj[:], lhsT=ceT[:], rhs=wcemb[:],
                             start=True, stop=True)
            proj_sb = bpool.tile([N_COUNTRY, 256], f32, tag="proj")
            nc.any.tensor_copy(proj_sb[:], ps_proj[:])
            nc.sync.dma_start(wc_ar[15:53, :], proj_sb[:])

            # ---------------- arenas
            agg_c = apool.tile([128, NCC * OUT], f32)
            agg_p = apool.tile([128, NPC * OUT], f32)
            rev_src = apool.tile([128, NT_REV], i32)
            nc.sync.dma_start(rev_src[:], rev_src_d[:])
            rev_dstl = apool.tile([128, NT_REV], f32)
            nc.sync.dma_start(rev_dstl[:], rev_dstl_d[:])
            buys_src = apool.tile([128, NT_BUYS], i32)
            nc.sync.dma_start(buys_src[:], buys_src_d[:])
            buys_dstl = apool.tile([128, NT_BUYS], f32)
            nc.sync.dma_start(buys_dstl[:], buys_dstl_d[:])
            rcnt_c = apool.tile([128, NCC], f32)
            nc.sync.dma_start(rcnt_c[:], rcnt_c_d[:])
            rcnt_p = apool.tile([128, NPC], f32)
            nc.sync.dma_start(rcnt_p[:], rcnt_p_d[:])
            pid_sb = apool.tile([128, NPC], i32)
            nc.sync.dma_start(pid_sb[:], pid_d[:])

            # ---------------- phase A: edge aggregation
            def edge_phase(nt, st, src_sb, dstl_sb, table, rcnt_sb, arena,
                           n_chunks):
                for c in range(n_chunks):
                    ntc = int(nt[c])
                    base = int(st[c])
                    ps = eps.tile([128, OUT], f32, tag="eps")
                    for t in range(ntc):
                        s = base + t
                        v = vpool.tile([128, OUT], f32, tag="v")
                        nc.gpsimd.indirect_dma_start(
                            out=v[:], out_offset=None, in_=table[:],
                            in_offset=IOA(ap=src_sb[:, s:s + 1], axis=0))
                        soh = spool.tile([128, 128], f32, tag="soh")
                        nc.vector.tensor_scalar(
                            soh[:], iota128[:], dstl_sb[:, s:s + 1], None,
                            mybir.AluOpType.is_equal)
                        nc.tensor.matmul(ps[:], lhsT=soh[:], rhs=v[:],
                                         start=(t == 0), stop=(t == ntc - 1))
                    nc.vector.tensor_scalar(
                        arena[:, c * OUT:(c + 1) * OUT], ps[:],
                        rcnt_sb[:, c:c + 1], None, mybir.AluOpType.mult)

            edge_phase(nt_rev, st_rev, rev_src, rev_dstl, h_p_full,
                       rcnt_c, agg_c, NCC)
            edge_phase(nt_buys, st_buys, buys_src, buys_dstl, h_c_full,
                       rcnt_p, agg_p, NPC)

            # ---------------- phase B helpers
            def lstm_tail(pre, c_src, t, hc_out, cc_out):
                gi = bpool.tile([128, OUT], f32, tag="gi")
                nc.scalar.activation(gi[:], pre[:, 0:64], AF.Sigmoid)
                gf = bpool.tile([128, OUT], f32, tag="gf")
                nc.scalar.activation(gf[:], pre[:, 64:128], AF.Sigmoid)
                gg = bpool.tile([128, OUT], f32, tag="gg")
                nc.scalar.activation(gg[:], pre[:, 128:192], AF.Tanh)
                go = bpool.tile([128, OUT], f32, tag="go")
                nc.scalar.activation(go[:], pre[:, 192:256], AF.Sigmoid)
                ct = bpool.tile([128, OUT], f32, tag="ct")
                nc.sync.dma_start(ct[:], c_src[t * 128:(t + 1) * 128, :])
                fc = bpool.tile([128, OUT], f32, tag="fc")
                nc.any.tensor_mul(fc[:], gf[:], ct[:])
                ig = bpool.tile([128, OUT], f32, tag="ig")
                nc.any.tensor_mul(ig[:], gi[:], gg[:])
                cn = bpool.tile([128, OUT], f32, tag="cn")
                nc.any.tensor_add(cn[:], fc[:], ig[:])
                nc.scalar.dma_start(cc_out[t * 128:(t + 1) * 128, :], cn[:])
                th = bpool.tile([128, OUT], f32, tag="th")
                nc.scalar.activation(th[:], cn[:], AF.Tanh)
                hn = bpool.tile([128, OUT], f32, tag="hn")
                nc.any.tensor_mul(hn[:], go[:], th[:])
                nc.scalar.dma_start(hc_out[t * 128:(t + 1) * 128, :], hn[:])
                return hn

            def transpose_to(sb_tag, in_ap, rows):
                ps = tps.tile([rows, 128], f32, tag="tp")
                nc.tensor.transpose(ps[:], in_ap, ident[:])
                t_sb = bpool.tile([rows, 128], f32, tag=sb_tag)
                nc.vector.tensor_copy(t_sb[:], ps[:])
                return t_sb

            # ---------------- phase B: products (40 tiles)
            for t in range(NPC):
                xc = bpool.tile([128, 22], f32, tag="xp")
                nc.sync.dma_start(xc[:, 0:5], x_p_sh[t * 128:(t + 1) * 128, :])
                nc.gpsimd.indirect_dma_start(
                    out=xc[:, 5:21], out_offset=None, in_=desc_emb_d[:],
                    in_offset=IOA(ap=pid_sb[:, t:t + 1], axis=0))
                nc.vector.memset(xc[:, 4:5], 1.0)
                nc.vector.memset(xc[:, 21:22], 1.0)
                xT = transpose_to("xpT", xc[:], 22)
                aT = transpose_to("aT", agg_p[:, t * OUT:(t + 1) * OUT], OUT)
                ht = bpool.tile([128, OUT], f32, tag="hload")
                nc.sync.dma_start(ht[:], h_p_sh[t * 128:(t + 1) * 128, :])
                hT = transpose_to("hT", ht[:], OUT)
                pre = pps.tile([128, 256], f32, tag="pre")
                nc.tensor.matmul(pre[:], lhsT=xT[:], rhs=wp[:],
                                 start=True, stop=False)
                nc.tensor.matmul(pre[:], lhsT=aT[:], rhs=wl_p[:],
                                 start=False, stop=False)
                nc.tensor.matmul(pre[:], lhsT=hT[:], rhs=wr_p[:],
                                 start=False, stop=True)
                lstm_tail(pre, c_p_sh, t, hp_o, cp_o)

            # ---------------- phase B: customers (196 tiles)
            for t in range(NCC):
                xc = bpool.tile([128, 54], f32, tag="xcat")
                nc.sync.dma_start(xc[:, 0:15],
                                  x_c_sh[t * 128:(t + 1) * 128, :])
                nc.any.tensor_scalar(
                    xc[:, 15:53], iota128[:, 0:N_COUNTRY], xc[:, 14:15],
                    None, mybir.AluOpType.is_equal)
                nc.vector.memset(xc[:, 14:15], 1.0)
                nc.vector.memset(xc[:, 53:54], 1.0)
                xT = transpose_to("xcT", xc[:], 54)
                aT = transpose_to("aT", agg_c[:, t * OUT:(t + 1) * OUT], OUT)
                ht = bpool.tile([128, OUT], f32, tag="hload")
                nc.sync.dma_start(ht[:], h_c_sh[t * 128:(t + 1) * 128, :])
                hT = transpose_to("hT", ht[:], OUT)
                pre = pps.tile([128, 256], f32, tag="pre")
                nc.tensor.matmul(pre[:], lhsT=xT[:], rhs=wc_ar[:],
                                 start=True, stop=False)
                nc.tensor.matmul(pre[:], lhsT=aT[:], rhs=wl_c[:],
                                 start=False, stop=False)
                nc.tensor.matmul(pre[:], lhsT=hT[:], rhs=wr_c[:],
                                 start=False, stop=True)
                hn = lstm_tail(pre, c_c_sh, t, hc_o, cc_o)
                # prediction MLP
                hT2 = transpose_to("hT2", hn[:], OUT)
                psR = tps.tile([32, 128], f32, tag="tp")
                nc.tensor.matmul(psR[:], lhsT=w1[:], rhs=hT2[:],
                                 start=True, stop=True)
                r = bpool.tile([33, 128], f32, tag="r")
                nc.scalar.activation(r[0:32, :], psR[:], AF.Relu,
                                     bias=b1[:, 0:1])
                nc.vector.memset(r[32:33, :], 1.0)
                psP = tps.tile([128, 1], f32, tag="tp")
                nc.tensor.matmul(psP[:], lhsT=r[:], rhs=w2b[:],
                                 start=True, stop=True)
                pt = bpool.tile([128, 1], f32, tag="pt")
                nc.any.tensor_copy(pt[:], psP[:])
                nc.scalar.dma_start(pred_o[t * 128:(t + 1) * 128, :], pt[:])

    return nc


# ---------------------------------------------------------------- kernel
def kernel(x_customer, x_product, h_customer, h_product, c_customer,
           c_product, params, src_buys, dst_buys, src_rev, dst_rev):
    x_customer = np.asarray(x_customer, np.float32)
    x_product = np.asarray(x_product, np.float32)
    h_customer = np.asarray(h_customer, np.float32)
    h_product = np.asarray(h_product, np.float32)
    c_customer = np.asarray(c_customer, np.float32)
    c_product = np.asarray(c_product, np.float32)
    src_buys = np.asarray(src_buys).astype(np.int64)
    dst_buys = np.asarray(dst_buys).astype(np.int64)
    src_rev = np.asarray(src_rev).astype(np.int64)
    dst_rev = np.asarray(dst_rev).astype(np.int64)

    # ---- edge prep (indices only)
    rev_pc, nt_rev, st_rev, rcnt_c_pc = _prep_edges(
        src_rev, dst_rev, NCS, NCC)
    buys_pc, nt_buys, st_buys, rcnt_p_pc = _prep_edges(
        src_buys, dst_buys, NPS, NPC)

    # ---- weights (pure relayout)
    P = params
    W = P["W"]
    b = P["b"]
    sage = P["sage"]
    wc_full = _stack_g([W[g]["customer"] for g in GATES])     # [18, 256]
    wp_full_w = _stack_g([W[g]["product"] for g in GATES])    # [20, 256]
    b_c = _stack_g([b[g]["customer"] for g in GATES])         # [1, 256]
    b_p = _stack_g([b[g]["product"] for g in GATES])
    bl_c = _stack_g([np.asarray(sage[g]["rev"]["bl"])[None, :]
                     for g in GATES])
    bl_p = _stack_g([np.asarray(sage[g]["buys"]["bl"])[None, :]
                     for g in GATES])
    wl_c = _stack_g([sage[g]["rev"]["Wl"] for g in GATES])
    wr_c = _stack_g([sage[g]["rev"]["Wr"] for g in GATES])
    wl_p = _stack_g([sage[g]["buys"]["Wl"] for g in GATES])
    wr_p = _stack_g([sage[g]["buys"]["Wr"] for g in GATES])
    wc_a = np.concatenate([wc_full[:14], b_c], axis=0)        # [15, 256]
    wc_emb = wc_full[14:18]                                   # [4, 256]
    wp_all = np.concatenate([wp_full_w[:4], b_p, wp_full_w[4:20], bl_p],
                            axis=0)                           # [22, 256]
    w1 = np.asarray(P["pred"]["W1"], np.float32)
    b1 = np.asarray(P["pred"]["b1"], np.float32).reshape(32, 1)
    w2b = np.concatenate(
        [np.asarray(P["pred"]["W2"], np.float32),
         np.asarray(P["pred"]["b2"], np.float32).reshape(1, 1)], axis=0)

    # ---- node shards (zero-padded)
    def pad_rows(a, n):
        out = np.zeros((n, a.shape[1]), np.float32)
        out[:a.shape[0]] = a
        return out

    h_c_full = pad_rows(h_customer, NC_PAD)
    h_p_full = pad_rows(h_product, NP_PAD)
    x_c_pad = pad_rows(x_customer, NC_PAD)
    x_p_pad = pad_rows(x_product, NP_PAD)
    c_c_pad = pad_rows(c_customer, NC_PAD)
    c_p_pad = pad_rows(c_product, NP_PAD)
    pid_all = x_p_pad[:, 4].astype(np.int32)

    # ---- build + compile (cached on edge structure)
    key = (int(st_rev[-1]), int(st_buys[-1]),
           tuple(nt_rev.tolist()), tuple(nt_buys.tolist()))
    if key not in _CACHE:
        nc = _build_nc(nt_rev, st_rev, nt_buys, st_buys)
        _CACHE[key] = _Runner(nc, NCORES)
    runner = _CACHE[key]

    in_maps = []
    for k in range(NCORES):
        cs, ce = k * NCS, (k + 1) * NCS
        ps_, pe_ = k * NPS, (k + 1) * NPS
        in_maps.append(dict(
            h_c_full=h_c_full, h_p_full=h_p_full,
            h_c_sh=h_c_full[cs:ce], h_p_sh=h_p_full[ps_:pe_],
            c_c_sh=c_c_pad[cs:ce], c_p_sh=c_p_pad[ps_:pe_],
            x_c_sh=x_c_pad[cs:ce], x_p_sh=x_p_pad[ps_:pe_],
            pid=np.ascontiguousarray(
                pid_all[ps_:pe_].reshape(-1, 128).T),
            desc_emb=np.asarray(P["desc_emb"], np.float32),
            country_emb=np.asarray(P["country_emb"], np.float32),
            rev_src=rev_pc[k][0], rev_dstl=rev_pc[k][1],
            rcnt_c=rcnt_c_pc[k],
            buys_src=buys_pc[k][0], buys_dstl=buys_pc[k][1],
            rcnt_p=rcnt_p_pc[k],
            wc_a=wc_a, wc_emb=wc_emb, bl_c=bl_c,
            wl_c=wl_c, wr_c=wr_c,
            wp=wp_all, wl_p=wl_p, wr_p=wr_p,
            w1=w1, b1=b1, w2b=w2b,
        ))

    global _LAST_IN_MAPS
    _LAST_IN_MAPS = in_maps
    res = runner.run(in_maps)

    # ---- unshard
    def cat(name, n_real, rows):
        return np.concatenate([res[k][name] for k in range(NCORES)],
                              axis=0)[:n_real]

    pred = cat("pred_o", NC, NCS)
    hc = cat("hc_o", NC, NCS)
    cc = cat("cc_o", NC, NCS)
    hp = cat("hp_o", NP, NPS)
    cp = cat("cp_o", NP, NPS)
    return pred, hc, hp, cc, cp
